# revision 1
# baseline (speedup 1.0000x reference)
"""Overlapping-chunk extraction kernel for Trainium2 (Bass).

Computes out[b, j, c, f] = x[b, 125*j + c, f] for j in [0, 255), c in [0, 250),
i.e. 255 half-overlapping chunks of length 250 from a (16, 32000, 64) signal.

Strategy (pure data movement, memory-bound):
  - Shard batch across 8 cores: 2 samples per core.
  - Per sample: ONE direct HBM->HBM DMA. Source = overlapping strided view
    (255 blocks of 16000 fp32 at stride 8000). Destination = fully contiguous
    output sample. A single sequential HBM write stream is the key to
    throughput on this part; measured ~4.7x faster than SBUF-staged variants
    with strided writes.
"""

import numpy as np

import concourse.bass as bass
import concourse.mybir as mybir
from concourse.bass_utils import run_bass_kernel_spmd

# Problem shape (hardcoded per contract)
B, T, F = 16, 32000, 64
N_CORES = 8
S = B // N_CORES          # samples per core = 2
NFC = 128                 # non-overlapping chunks per sample
CHUNK = 250               # frames per chunk
NOV = 2 * NFC - 1         # 255 overlapped output chunks
PART_FREE = CHUNK * F     # 16000 fp32 per chunk
HALF_FREE = PART_FREE // 2  # 8000 fp32 = 125 frames (chunk advance)
SAMPLE_IN = T * F         # 2_048_000 fp32 per input sample
SAMPLE_OUT = NOV * PART_FREE  # 4_080_000 fp32 per output sample

_NC_CACHE = {}


def _build_module():
    nc = bass.Bass(trn_type="TRN2")
    x = nc.dram_tensor("x", [S, T, F], mybir.dt.float32, kind="ExternalInput")
    y = nc.dram_tensor(
        "y", [S, NOV, CHUNK, F], mybir.dt.float32, kind="ExternalOutput"
    )
    x_t = x[:, :, :].tensor
    y_t = y[:, :, :, :].tensor

    with (
        nc.semaphore("st") as st,
        nc.Block() as block,
    ):
        @block.gpsimd
        def _(gpsimd):
            with nc.allow_non_contiguous_dma(reason="overlapping chunk reads"):
                for s in range(S):
                    src = bass.AP(
                        x_t, s * SAMPLE_IN, [[HALF_FREE, NOV], [1, PART_FREE]]
                    )
                    dst = bass.AP(
                        y_t, s * SAMPLE_OUT, [[PART_FREE, NOV], [1, PART_FREE]]
                    )
                    gpsimd.dma_start(dst, src).then_inc(st, 16)
                gpsimd.wait_ge(st, 16 * S)

    return nc


def get_module():
    if "nc" not in _NC_CACHE:
        _NC_CACHE["nc"] = _build_module()
    return _NC_CACHE["nc"]


def kernel(x):
    x = np.ascontiguousarray(np.asarray(x), dtype=np.float32)
    assert x.shape == (B, T, F), x.shape
    nc = get_module()
    in_maps = [{"x": x[i * S : (i + 1) * S]} for i in range(N_CORES)]
    res = run_bass_kernel_spmd(nc, in_maps, core_ids=list(range(N_CORES)))
    return np.concatenate([r["y"] for r in res.results], axis=0)



# revision 2
# speedup vs baseline: 51.4227x; 51.4227x over previous
"""Overlapping-chunk extraction kernel for Trainium2 (Bass).

Computes out[b, j, c, f] = x[b, 125*j + c, f] for j in [0, 255), c in [0, 250),
i.e. 255 half-overlapping chunks of length 250 from a (16, 32000, 64) signal.

Strategy (pure data movement, memory-bound):
  - Shard batch across 8 cores: 2 samples per core.
  - Per sample: ONE direct HBM->HBM DMA. Source = overlapping strided view
    (255 blocks of 16000 fp32 at stride 8000). Destination = fully contiguous
    output sample. The two per-sample DMAs run concurrently on the gpsimd
    (SWDGE) queue.

Alternatives measured slower on this axon/PJRT runtime (repeat-count
wall-clock differencing, 8-core SPMD; this kernel ~160-230 us/core,
~290 GB/s):
  - contiguous-read/strided-write HBM->HBM mirror: +25%.
  - SBUF staging (cuts HBM traffic 65->49 MB/core): 6x slower — HBM->SBUF
    loads run at ~42 GB/s here (~1.5 us/descriptor regardless of size);
    strided SBUF->HBM stores at ~23 GB/s.
  - serializing the two DMAs: +30%; splitting into 4-8 DMAs: ~2x;
    single fused 3-dim DMA: +28%; HWDGE (sync/scalar) queues: ~2x;
    spreading across 2-3 queues: 1.3-4x.
"""

import numpy as np

import concourse.bass as bass
import concourse.mybir as mybir
from concourse.bass_utils import run_bass_kernel_spmd

# Problem shape (hardcoded per contract)
B, T, F = 16, 32000, 64
N_CORES = 8
S = B // N_CORES          # samples per core = 2
NFC = 128                 # non-overlapping chunks per sample
CHUNK = 250               # frames per chunk
NOV = 2 * NFC - 1         # 255 overlapped output chunks
PART_FREE = CHUNK * F     # 16000 fp32 per chunk
HALF_FREE = PART_FREE // 2  # 8000 fp32 = 125 frames (chunk advance)
SAMPLE_IN = T * F         # 2_048_000 fp32 per input sample
SAMPLE_OUT = NOV * PART_FREE  # 4_080_000 fp32 per output sample

_NC_CACHE = {}


def _build_module():
    nc = bass.Bass(trn_type="TRN2")
    x = nc.dram_tensor("x", [S, T, F], mybir.dt.float32, kind="ExternalInput")
    y = nc.dram_tensor(
        "y", [S, NOV, CHUNK, F], mybir.dt.float32, kind="ExternalOutput"
    )
    x_t = x[:, :, :].tensor
    y_t = y[:, :, :, :].tensor

    with (
        nc.semaphore("st") as st,
        nc.Block() as block,
    ):
        @block.gpsimd
        def _(gpsimd):
            with nc.allow_non_contiguous_dma(reason="overlapping chunk reads"):
                for s in range(S):
                    src = bass.AP(
                        x_t, s * SAMPLE_IN, [[HALF_FREE, NOV], [1, PART_FREE]]
                    )
                    dst = bass.AP(
                        y_t, s * SAMPLE_OUT, [[PART_FREE, NOV], [1, PART_FREE]]
                    )
                    gpsimd.dma_start(dst, src).then_inc(st, 16)
                gpsimd.wait_ge(st, 16 * S)

    return nc


def get_module():
    if "nc" not in _NC_CACHE:
        _NC_CACHE["nc"] = _build_module()
    return _NC_CACHE["nc"]


def kernel(x):
    x = np.ascontiguousarray(np.asarray(x), dtype=np.float32)
    assert x.shape == (B, T, F), x.shape
    nc = get_module()
    in_maps = [{"x": x[i * S : (i + 1) * S]} for i in range(N_CORES)]
    res = run_bass_kernel_spmd(nc, in_maps, core_ids=list(range(N_CORES)))
    return np.concatenate([r["y"] for r in res.results], axis=0)



# revision 3
# speedup vs baseline: 52.8607x; 1.0280x over previous
"""Overlapping-chunk extraction kernel for Trainium2 (Bass).

Computes out[b, j, c, f] = x[b, 125*j + c, f] for j in [0, 255), c in [0, 250),
i.e. 255 half-overlapping chunks of length 250 from a (16, 32000, 64) signal.

Strategy (pure data movement, memory-bound):
  - Shard batch across 8 cores: 2 samples per core.
  - Per sample: ONE direct HBM->HBM DMA. Source = overlapping strided view
    (255 blocks of 16000 fp32 at stride 8000). Destination = fully contiguous
    output sample. The two per-sample DMAs run concurrently on the gpsimd
    (SWDGE) queue.

Alternatives measured slower on this axon/PJRT runtime (repeat-count
wall-clock differencing, 8-core SPMD; this kernel ~256 us/core at R=201
where min/median estimators agree, ~255 GB/s effective vs the 182 us
HBM-per-NC floor for the 65.3 MB/core of HBM traffic):
  - contiguous-read/strided-write HBM->HBM mirror: +25%.
  - SBUF staging (cuts HBM traffic 65->49 MB/core): 6x slower — HBM->SBUF
    loads run at ~42 GB/s here (~1.5 us/descriptor regardless of size);
    strided SBUF->HBM stores at ~23 GB/s.
  - serializing the two DMAs: +30%; splitting into 4-8 DMAs: ~2x;
    single fused 3-dim DMA: +28%; HWDGE (sync/scalar) queues: ~2x;
    spreading across 2-3 queues: 1.3-4x.
"""

import numpy as np

import concourse.bass as bass
import concourse.mybir as mybir
from concourse.bass_utils import run_bass_kernel_spmd

# Problem shape (hardcoded per contract)
B, T, F = 16, 32000, 64
N_CORES = 8
S = B // N_CORES          # samples per core = 2
NFC = 128                 # non-overlapping chunks per sample
CHUNK = 250               # frames per chunk
NOV = 2 * NFC - 1         # 255 overlapped output chunks
PART_FREE = CHUNK * F     # 16000 fp32 per chunk
HALF_FREE = PART_FREE // 2  # 8000 fp32 = 125 frames (chunk advance)
SAMPLE_IN = T * F         # 2_048_000 fp32 per input sample
SAMPLE_OUT = NOV * PART_FREE  # 4_080_000 fp32 per output sample

_NC_CACHE = {}


def _build_module():
    nc = bass.Bass(trn_type="TRN2")
    x = nc.dram_tensor("x", [S, T, F], mybir.dt.float32, kind="ExternalInput")
    y = nc.dram_tensor(
        "y", [S, NOV, CHUNK, F], mybir.dt.float32, kind="ExternalOutput"
    )
    x_t = x[:, :, :].tensor
    y_t = y[:, :, :, :].tensor

    with (
        nc.semaphore("st") as st,
        nc.Block() as block,
    ):
        @block.gpsimd
        def _(gpsimd):
            with nc.allow_non_contiguous_dma(reason="overlapping chunk reads"):
                for s in range(S):
                    src = bass.AP(
                        x_t, s * SAMPLE_IN, [[HALF_FREE, NOV], [1, PART_FREE]]
                    )
                    dst = bass.AP(
                        y_t, s * SAMPLE_OUT, [[PART_FREE, NOV], [1, PART_FREE]]
                    )
                    gpsimd.dma_start(dst, src).then_inc(st, 16)
                gpsimd.wait_ge(st, 16 * S)

    return nc


def get_module():
    if "nc" not in _NC_CACHE:
        _NC_CACHE["nc"] = _build_module()
    return _NC_CACHE["nc"]


def kernel(x):
    x = np.ascontiguousarray(np.asarray(x), dtype=np.float32)
    assert x.shape == (B, T, F), x.shape
    nc = get_module()
    in_maps = [{"x": x[i * S : (i + 1) * S]} for i in range(N_CORES)]
    res = run_bass_kernel_spmd(nc, in_maps, core_ids=list(range(N_CORES)))
    return np.concatenate([r["y"] for r in res.results], axis=0)

